# revision 61
# baseline (speedup 1.0000x reference)
"""Dice-loss kernel for Trainium2 (Bass/Tile), 8-core data-parallel SPMD.

Strategy
--------
reference: pred = argmax_c(logits); for c in 1..4:
    inter_c = #{v : pred[v]==c and tgt[v]==c},  tsum_c = #{v : tgt[v]==c}
    dice_c = (2*inter_c + eps) / (inter_c + tsum_c + eps); loss = 1 - mean(dice)

The voxel axis (B*D*H*W = 7,077,888) is sharded 8 ways; each core gets
[5, 128, 6912] fp16 logits and [128, 6912] fp16 labels.  On device (all DVE):
  m   = max of the 5 class planes            (4 tensor_tensor max)
  e_c = (l_c >= m)                           (4 tensor_tensor is_ge)
  t_c = (tgt == c)  + free-axis sum -> tsum  (4 tensor_scalar, 4x mode)
  i_c = t_c * e_c   + free-axis sum -> inter (4 scalar_tensor_tensor)
Per-partition partial sums [128, 8] go back to the host, which reduces
across partitions/cores and evaluates the scalar dice in float32.

fp16 note: logits are converted to fp16 on the host.  argmax ties after
fp16 rounding affect ~0.03% of voxels, giving ~1e-4 relative error on the
loss (the check tolerance is far looser).  Counts stay exact integers in
fp32 accumulators.
"""

import sys
from contextlib import ExitStack

import numpy as np

for _p in ("/opt/trn_rl_repo", "/opt/pypackages"):
    if _p not in sys.path:
        sys.path.append(_p)

import concourse.bacc as bacc
import concourse.bass as bass
import concourse.tile as tile
from concourse import mybir
from concourse.bass_utils import run_bass_kernel_spmd

# Problem shape (hardcoded per contract: kernel.py must be self-contained).
B, C, D, H, W = 2, 5, 96, 192, 192
N_CORES = 8
P = 128                      # SBUF partitions
NVOX = B * D * H * W         # 7,077,888 voxels
SHARD = NVOX // N_CORES      # 884,736 voxels per core
FTOT = SHARD // P            # 6,912 free elems per partition
# Uneven tiling: small first tile starts compute sooner, small last tile
# shortens the PE/ACT tail.  All multiples of 128 (PE chunking).
TILES = [128, 1280, 1152, 2432, 1664, 256]
# Tiles whose tsum / inter_1,2 reductions go to PE ones-matmuls instead of
# ACT, so ACT drains before DVE finishes (kills the ACT tail).  Must form a
# suffix, and the first such tile must have a >=512 first chunk (PSUM zero
# rule).
PE_TSUM_FROM = 4
NT = len(TILES)
NCLS = C - 1                 # foreground classes 1..4
NQ = 2 * NCLS                # 4 inter + 4 tsum accumulators
EPS = 1e-8
assert sum(TILES) == FTOT


def emit_dice_kernel(
    tc,
    logits_ap,
    tgt_ap,
    partials_ap,
    cms_ap,
    n_cls,
    p,
    tiles,
    pe_tsum_from=None,
    tsums_ap=None,
):
    """Emit the per-core dice partial-sums program into TileContext `tc`.

    logits_ap:   DRAM [C, p, ftot] fp16
    tgt_ap:      DRAM [p, ftot]    fp16 (labels 0..C-1, exact)
    partials_ap: DRAM [p, 6*nt]    f32 -- ACT accum columns, layout q*nt + i
                 with q in {inter_1, inter_2, tsum_1, tsum_2, tsum_3, tsum_4}
    cms_ap:      DRAM [p, 256]     f32 -- PE confusion blocks: cols 0:128 are
                 sum_chunks t_3^T e_3, cols 128:256 the same for class 4; the
                 host takes the trace (diagonal sum) to get inter_3/inter_4.
    tiles:       list of free-dim tile sizes, each a multiple of 128 (PE
                 chunking).  Small first tile starts compute sooner, small
                 last tile shortens the ACT/PE tail after DVE finishes.

    DVE: max tree, is_ge (5 classes), is_eq (4), mult (classes 1,2 only).
    ACT: 6 plane-sums/tile via copy-accum.
    PE:  inter_3/inter_4 as accumulated t^T e [128,128] blocks (product and
         voxel reduction fused into the matmul).
    """
    nc = tc.nc
    n_cls_total = n_cls + 1  # C
    nt = len(tiles)
    fdmax = max(tiles)
    fp16 = mybir.dt.float16
    f32 = mybir.dt.float32
    Alu = mybir.AluOpType
    Act = mybir.ActivationFunctionType
    assert all(fd % 128 == 0 for fd in tiles)
    if pe_tsum_from is None:
        pe_tsum_from = nt  # all tsums on ACT
    ts_w = min(512, max(tiles[pe_tsum_from:], default=512))

    def chunk_list(fd, w):
        out, off = [], 0
        while off < fd:
            ww = min(w, fd - off)
            out.append((off, ww))
            off += ww
        return out

    with ExitStack() as ctx:
        pool_in = ctx.enter_context(tc.tile_pool(name="in", bufs=2))
        # the target plane is small; a third buffer stops tile i+2's tg DMA
        # from waiting on tile i's readers
        pool_tg = ctx.enter_context(tc.tile_pool(name="tgp", bufs=3))
        pool_t1 = ctx.enter_context(tc.tile_pool(name="t1", bufs=1))
        pool_t2 = ctx.enter_context(tc.tile_pool(name="t2", bufs=2))
        pool_acc = ctx.enter_context(tc.tile_pool(name="acc", bufs=1))
        pool_ps = ctx.enter_context(tc.tile_pool(name="ps", bufs=1, space="PSUM"))

        acc = pool_acc.tile([p, 6 * nt], f32, tag="acc")
        nc.vector.memset(acc, 0.0)
        ones = pool_acc.tile([p, 1], fp16, tag="ones")
        nc.vector.memset(ones, 1.0)
        # 2 PSUM confusion blocks: class 3, class 4
        cm = [
            pool_ps.tile([128, 128], f32, tag=f"cm{q}", name=f"cm{q}")
            for q in range(2)
        ]
        # 6 PSUM rows for late-tile ones-reductions:
        # rows 0..3 = tsum_1..4, rows 4..5 = inter_1, inter_2
        tsp = [
            pool_ps.tile([1, ts_w], f32, tag=f"tsp{q}", name=f"tsp{q}")
            for q in range(6)
        ]

        base = 0
        for i, fd in enumerate(tiles):
            sl = slice(base, base + fd)
            base += fd
            # target first: the t_c tensor_scalar ops need it early.  Logits
            # for classes 1-4 land in one 4-plane tile (a single fused is_ge
            # covers them).  Tile 1 is deadline-critical (it's needed right
            # after the tiny tile 0), so its transfers are split across more
            # HWDGE queues; later tiles have a full tile of slack and ride
            # one grouped transfer each.
            tg = pool_tg.tile([p, fdmax], fp16, tag="tg")
            lgf = pool_in.tile([p, 4, fdmax], fp16, tag="lgf")
            lg0 = pool_in.tile([p, fdmax], fp16, tag="lg0")
            nc.sync.dma_start(out=tg[:, 0:fd], in_=tgt_ap[:, sl])
            nc.sync.dma_start(
                out=lgf[:, :, 0:fd],
                in_=logits_ap[1:n_cls_total, :, sl].rearrange("c p f -> p c f"),
            )
            nc.sync.dma_start(out=lg0[:, 0:fd], in_=logits_ap[0, :, sl])

            # m = max over the 5 class planes: 3 TT ops (first one covers two
            # plane-pairs in a single instruction)
            mab = pool_t1.tile([p, 2, fdmax], fp16, tag="mab")
            m = pool_t1.tile([p, fdmax], fp16, tag="m")
            nc.vector.tensor_tensor(
                mab[:, :, 0:fd], lgf[:, 0:2, 0:fd], lgf[:, 2:4, 0:fd], Alu.max
            )
            nc.vector.tensor_tensor(
                m[:, 0:fd], mab[:, 0, 0:fd], mab[:, 1, 0:fd], Alu.max
            )
            nc.vector.tensor_tensor(m[:, 0:fd], m[:, 0:fd], lg0[:, 0:fd], Alu.max)

            # e = (l_c >= m) for all 4 foreground classes in ONE op, with m
            # broadcast along the class dim via a step-0 AP
            ev = pool_t2.tile([p, 4, fdmax], fp16, tag="ev")
            m_sl = m[:, 0:fd]
            m_bc = bass.AP(
                tensor=m_sl.tensor,
                offset=m_sl.offset,
                ap=[list(m_sl.ap[0]), [0, 4], list(m_sl.ap[1])],
            )
            nc.vector.tensor_tensor(ev[:, :, 0:fd], lgf[:, :, 0:fd], m_bc, Alu.is_ge)

            tv = pool_t2.tile([p, 4, fdmax], fp16, tag="tv")
            dump = pool_t1.tile([p, fdmax], fp16, tag="dump")
            # PE classes first: the matmul chain is the tightest engine and
            # needs its inputs as early as possible.  In the last tile, ACT/PE
            # consumers come first so they finish with DVE.
            order = (1, 2, 3, 4) if i == nt - 1 else (3, 4, 1, 2)
            for c in order:
                cls_i = c - 1  # 0..3
                nc.vector.tensor_scalar(
                    tv[:, cls_i, 0:fd], tg[:, 0:fd], float(c), None, Alu.is_equal
                )
            a12 = pool_t2.tile([p, 2, fdmax], fp16, tag="a12")
            nc.vector.tensor_tensor(
                a12[:, :, 0:fd], tv[:, 0:2, 0:fd], ev[:, 0:2, 0:fd], Alu.mult
            )

            for c in order:
                cls_i = c - 1  # 0..3
                on_pe = cls_i >= 2
                e = ev[:, cls_i]
                t_c = tv[:, cls_i]
                if on_pe:
                    # inter_c: accumulate t^T e blocks on PE (fused mult+reduce)
                    first = i == 0
                    last = i == nt - 1
                    nchunks = fd // 128
                    for k in range(nchunks):
                        o = k * 128
                        nc.tensor.matmul(
                            cm[cls_i - 2],
                            t_c[:, o : o + 128],
                            e[:, o : o + 128],
                            start=(first and k == 0),
                            stop=(last and k == nchunks - 1),
                        )
                else:
                    if i < pe_tsum_from:
                        # inter cols q=cls_i on ACT (early tiles)
                        nc.scalar.activation(
                            dump[:, 0:fd],
                            a12[:, cls_i, 0:fd],
                            Act.Copy,
                            accum_out=acc[:, cls_i * nt + i : cls_i * nt + i + 1],
                        )
                    else:
                        for k, (o, w) in enumerate(chunk_list(fd, ts_w)):
                            nc.tensor.matmul(
                                tsp[4 + cls_i][:, 0:w],
                                ones,
                                a12[:, cls_i, o : o + w],
                                start=(i == pe_tsum_from and k == 0),
                                stop=(i == nt - 1 and o + w == fd),
                            )
                if i < pe_tsum_from:
                    # tsum cols q=2+cls_i on ACT (early tiles)
                    nc.scalar.activation(
                        dump[:, 0:fd],
                        t_c[:, 0:fd],
                        Act.Copy,
                        accum_out=acc[
                            :, (2 + cls_i) * nt + i : (2 + cls_i) * nt + i + 1
                        ],
                    )
                else:
                    # late tiles: tsum via PE ones-matmul so ACT drains early
                    for k, (o, w) in enumerate(chunk_list(fd, ts_w)):
                        nc.tensor.matmul(
                            tsp[cls_i][:, 0:w],
                            ones,
                            t_c[:, o : o + w],
                            start=(i == pe_tsum_from and k == 0),
                            stop=(
                                i == nt - 1 and o + w == fd
                            ),
                        )

        nc.sync.dma_start(out=partials_ap, in_=acc)
        # PSUM is not DMA-able: stage the confusion blocks (DVE) and the
        # tsum rows (ACT) through SBUF.
        cmout = pool_acc.tile([p, 256], f32, tag="cmout")
        nc.vector.tensor_copy(cmout[:, 0:128], cm[0])
        nc.vector.tensor_copy(cmout[:, 128:256], cm[1])
        nc.sync.dma_start(out=cms_ap, in_=cmout)
        if pe_tsum_from < nt:
            # split the staging copies across DVE and ACT (both idle here)
            tsout = pool_acc.tile([1, 6 * ts_w], f32, tag="tsout")
            for q in range(6):
                dst = tsout[:, q * ts_w : (q + 1) * ts_w]
                # DVE also stages the two cm blocks, so give it only 2 of 6
                if q < 2:
                    nc.vector.tensor_copy(dst, tsp[q])
                else:
                    nc.scalar.activation(dst, tsp[q], Act.Copy)
            nc.sync.dma_start(out=tsums_ap, in_=tsout)


_PROGRAM_CACHE = {}


def build_program():
    key = (C, P, FTOT, tuple(TILES))
    if key in _PROGRAM_CACHE:
        return _PROGRAM_CACHE[key]
    nc = bacc.Bacc("TRN2", debug=False, target_bir_lowering=False)
    logits = nc.dram_tensor(
        "logits", [C, P, FTOT], mybir.dt.float16, kind="ExternalInput"
    )
    tgt = nc.dram_tensor("tgt", [P, FTOT], mybir.dt.float16, kind="ExternalInput")
    partials = nc.dram_tensor(
        "partials", [P, 6 * NT], mybir.dt.float32, kind="ExternalOutput"
    )
    cms = nc.dram_tensor("cms", [P, 256], mybir.dt.float32, kind="ExternalOutput")
    tsums = nc.dram_tensor(
        "tsums", [1, 6 * 512], mybir.dt.float32, kind="ExternalOutput"
    )
    with tile.TileContext(nc) as tc:
        emit_dice_kernel(
            tc,
            logits.ap(),
            tgt.ap(),
            partials.ap(),
            cms.ap(),
            NCLS,
            P,
            TILES,
            pe_tsum_from=PE_TSUM_FROM,
            tsums_ap=tsums.ap(),
        )
    nc.compile()
    _PROGRAM_CACHE[key] = nc
    return nc


def make_in_maps(input2, target1):
    lg16 = np.asarray(input2, dtype=np.float32).astype(np.float16)
    tg16 = np.asarray(target1).astype(np.float16)
    lgf = lg16.reshape(B, C, NVOX // B)
    tgf = tg16.reshape(B, NVOX // B)
    shards_per_b = N_CORES // B
    s = (NVOX // B) // shards_per_b
    in_maps = []
    for core in range(N_CORES):
        b, q = divmod(core, shards_per_b)
        sl = slice(q * s, (q + 1) * s)
        in_maps.append(
            {
                "logits": np.ascontiguousarray(lgf[b, :, sl]).reshape(C, P, FTOT),
                "tgt": np.ascontiguousarray(tgf[b, sl]).reshape(P, FTOT),
            }
        )
    return in_maps


def _finish(results):
    """Host-side reduction of per-core partials -> scalar loss (float32).

    partials [P, 6*NT]: cols q*NT+i, q in {inter_1, inter_2, tsum_1..tsum_4};
    cms [P, 256]: accumulated t^T e blocks for classes 3, 4 -- trace = inter.
    """
    total = np.zeros(NQ, dtype=np.float64)  # inter_1..4, tsum_1..4
    for r in results:
        pa = r["partials"].astype(np.float64).reshape(P, 6, NT).sum(axis=(0, 2))
        cms = r["cms"].astype(np.float64)
        ts = r["tsums"].astype(np.float64).reshape(6, 512).sum(axis=1)
        total[0] += pa[0] + ts[4]  # inter_1
        total[1] += pa[1] + ts[5]  # inter_2
        total[2] += np.trace(cms[:, 0:128])  # inter_3
        total[3] += np.trace(cms[:, 128:256])  # inter_4
        total[4] += pa[2] + ts[0]  # tsum_1
        total[5] += pa[3] + ts[1]  # tsum_2
        total[6] += pa[4] + ts[2]  # tsum_3
        total[7] += pa[5] + ts[3]  # tsum_4
    inter = total[:NCLS].astype(np.float32)
    tsum = total[NCLS:].astype(np.float32)
    eps = np.float32(EPS)
    dice = (np.float32(2.0) * inter + eps) / (inter + tsum + eps)
    loss = np.float32(1.0) - np.mean(dice, dtype=np.float32)
    return np.array([loss], dtype=np.float32)


# test.py can set e.g. RUN_KWARGS.update(trace=True) to profile; the grader
# path leaves this empty.
RUN_KWARGS = {}
LAST_RESULT = None


def kernel(input2, target1):
    global LAST_RESULT
    nc = build_program()
    in_maps = make_in_maps(input2, target1)
    res = run_bass_kernel_spmd(nc, in_maps, core_ids=list(range(N_CORES)), **RUN_KWARGS)
    LAST_RESULT = res
    return _finish(res.results)


# revision 62
# speedup vs baseline: 1.0218x; 1.0218x over previous
"""Dice-loss kernel for Trainium2 (Bass/Tile), 8-core data-parallel SPMD.

Strategy
--------
reference: pred = argmax_c(logits); for c in 1..4:
    inter_c = #{v : pred[v]==c and tgt[v]==c},  tsum_c = #{v : tgt[v]==c}
    dice_c = (2*inter_c + eps) / (inter_c + tsum_c + eps); loss = 1 - mean(dice)

The voxel axis (B*D*H*W = 7,077,888) is sharded 8 ways; each core gets
[5, 128, 6912] fp16 logits and [128, 6912] fp16 labels.  On device (all DVE):
  m   = max of the 5 class planes            (4 tensor_tensor max)
  e_c = (l_c >= m)                           (4 tensor_tensor is_ge)
  t_c = (tgt == c)  + free-axis sum -> tsum  (4 tensor_scalar, 4x mode)
  i_c = t_c * e_c   + free-axis sum -> inter (4 scalar_tensor_tensor)
Per-partition partial sums [128, 8] go back to the host, which reduces
across partitions/cores and evaluates the scalar dice in float32.

fp16 note: logits are converted to fp16 on the host.  argmax ties after
fp16 rounding affect ~0.03% of voxels, giving ~1e-4 relative error on the
loss (the check tolerance is far looser).  Counts stay exact integers in
fp32 accumulators.
"""

import sys
from contextlib import ExitStack

import numpy as np

for _p in ("/opt/trn_rl_repo", "/opt/pypackages"):
    if _p not in sys.path:
        sys.path.append(_p)

import concourse.bacc as bacc
import concourse.bass as bass
import concourse.tile as tile
from concourse import mybir
from concourse.bass_utils import run_bass_kernel_spmd

# Problem shape (hardcoded per contract: kernel.py must be self-contained).
B, C, D, H, W = 2, 5, 96, 192, 192
N_CORES = 8
P = 128                      # SBUF partitions
NVOX = B * D * H * W         # 7,077,888 voxels
SHARD = NVOX // N_CORES      # 884,736 voxels per core
FTOT = SHARD // P            # 6,912 free elems per partition
# Uneven tiling: small first tile starts compute sooner, small last tile
# shortens the PE/ACT tail.  All multiples of 128 (PE chunking).
TILES = [128, 1280, 1152, 2432, 1664, 256]
# Tiles whose tsum / inter_1,2 reductions go to PE ones-matmuls instead of
# ACT, so ACT drains before DVE finishes (kills the ACT tail).  Must form a
# suffix, and the first such tile must have a >=512 first chunk (PSUM zero
# rule).
PE_TSUM_FROM = 4
NT = len(TILES)
NCLS = C - 1                 # foreground classes 1..4
NQ = 2 * NCLS                # 4 inter + 4 tsum accumulators
EPS = 1e-8
assert sum(TILES) == FTOT


def emit_dice_kernel(
    tc,
    logits_ap,
    tgt_ap,
    partials_ap,
    cms_ap,
    n_cls,
    p,
    tiles,
    pe_tsum_from=None,
    tsums_ap=None,
):
    """Emit the per-core dice partial-sums program into TileContext `tc`.

    logits_ap:   DRAM [C, p, ftot] fp16
    tgt_ap:      DRAM [p, ftot]    fp16 (labels 0..C-1, exact)
    partials_ap: DRAM [p, 6*nt]    f32 -- ACT accum columns, layout q*nt + i
                 with q in {inter_1, inter_2, tsum_1, tsum_2, tsum_3, tsum_4}
    cms_ap:      DRAM [p, 256]     f32 -- PE confusion blocks: cols 0:128 are
                 sum_chunks t_3^T e_3, cols 128:256 the same for class 4; the
                 host takes the trace (diagonal sum) to get inter_3/inter_4.
    tiles:       list of free-dim tile sizes, each a multiple of 128 (PE
                 chunking).  Small first tile starts compute sooner, small
                 last tile shortens the ACT/PE tail after DVE finishes.

    DVE: max tree, is_ge (5 classes), is_eq (4), mult (classes 1,2 only).
    ACT: 6 plane-sums/tile via copy-accum.
    PE:  inter_3/inter_4 as accumulated t^T e [128,128] blocks (product and
         voxel reduction fused into the matmul).
    """
    nc = tc.nc
    n_cls_total = n_cls + 1  # C
    nt = len(tiles)
    fdmax = max(tiles)
    fp16 = mybir.dt.float16
    f32 = mybir.dt.float32
    Alu = mybir.AluOpType
    Act = mybir.ActivationFunctionType
    assert all(fd % 128 == 0 for fd in tiles)
    if pe_tsum_from is None:
        pe_tsum_from = nt  # all tsums on ACT
    ts_w = min(512, max(tiles[pe_tsum_from:], default=512))

    def chunk_list(fd, w):
        out, off = [], 0
        while off < fd:
            ww = min(w, fd - off)
            out.append((off, ww))
            off += ww
        return out

    with ExitStack() as ctx:
        pool_in = ctx.enter_context(tc.tile_pool(name="in", bufs=2))
        pool_t1 = ctx.enter_context(tc.tile_pool(name="t1", bufs=1))
        pool_t2 = ctx.enter_context(tc.tile_pool(name="t2", bufs=2))
        pool_acc = ctx.enter_context(tc.tile_pool(name="acc", bufs=1))
        pool_ps = ctx.enter_context(tc.tile_pool(name="ps", bufs=1, space="PSUM"))

        acc = pool_acc.tile([p, 6 * nt], f32, tag="acc")
        nc.vector.memset(acc, 0.0)
        ones = pool_acc.tile([p, 1], fp16, tag="ones")
        nc.vector.memset(ones, 1.0)
        # 2 PSUM confusion blocks: class 3, class 4
        cm = [
            pool_ps.tile([128, 128], f32, tag=f"cm{q}", name=f"cm{q}")
            for q in range(2)
        ]
        # 6 PSUM rows for late-tile ones-reductions:
        # rows 0..3 = tsum_1..4, rows 4..5 = inter_1, inter_2
        tsp = [
            pool_ps.tile([1, ts_w], f32, tag=f"tsp{q}", name=f"tsp{q}")
            for q in range(6)
        ]

        base = 0
        for i, fd in enumerate(tiles):
            sl = slice(base, base + fd)
            base += fd
            # target first: the t_c tensor_scalar ops need it early.  Logits
            # for classes 1-4 land in one 4-plane tile (a single fused is_ge
            # covers them).  Tile 1 is deadline-critical (it's needed right
            # after the tiny tile 0), so its transfers are split across more
            # HWDGE queues; later tiles have a full tile of slack and ride
            # one grouped transfer each.
            tg = pool_in.tile([p, fdmax], fp16, tag="tg")
            lgf = pool_in.tile([p, 4, fdmax], fp16, tag="lgf")
            lg0 = pool_in.tile([p, fdmax], fp16, tag="lg0")
            nc.sync.dma_start(out=tg[:, 0:fd], in_=tgt_ap[:, sl])
            nc.sync.dma_start(
                out=lgf[:, :, 0:fd],
                in_=logits_ap[1:n_cls_total, :, sl].rearrange("c p f -> p c f"),
            )
            nc.sync.dma_start(out=lg0[:, 0:fd], in_=logits_ap[0, :, sl])

            # m = max over the 5 class planes: 3 TT ops (first one covers two
            # plane-pairs in a single instruction)
            mab = pool_t1.tile([p, 2, fdmax], fp16, tag="mab")
            m = pool_t1.tile([p, fdmax], fp16, tag="m")
            nc.vector.tensor_tensor(
                mab[:, :, 0:fd], lgf[:, 0:2, 0:fd], lgf[:, 2:4, 0:fd], Alu.max
            )
            nc.vector.tensor_tensor(
                m[:, 0:fd], mab[:, 0, 0:fd], mab[:, 1, 0:fd], Alu.max
            )
            nc.vector.tensor_tensor(m[:, 0:fd], m[:, 0:fd], lg0[:, 0:fd], Alu.max)

            # e = (l_c >= m) for all 4 foreground classes in ONE op, with m
            # broadcast along the class dim via a step-0 AP
            ev = pool_t2.tile([p, 4, fdmax], fp16, tag="ev")
            m_sl = m[:, 0:fd]
            m_bc = bass.AP(
                tensor=m_sl.tensor,
                offset=m_sl.offset,
                ap=[list(m_sl.ap[0]), [0, 4], list(m_sl.ap[1])],
            )
            nc.vector.tensor_tensor(ev[:, :, 0:fd], lgf[:, :, 0:fd], m_bc, Alu.is_ge)

            tv = pool_t2.tile([p, 4, fdmax], fp16, tag="tv")
            dump = pool_t1.tile([p, fdmax], fp16, tag="dump")
            # PE classes first: the matmul chain is the tightest engine and
            # needs its inputs as early as possible.  In the last tile, ACT/PE
            # consumers come first so they finish with DVE.
            order = (1, 2, 3, 4) if i == nt - 1 else (3, 4, 1, 2)
            for c in order:
                cls_i = c - 1  # 0..3
                nc.vector.tensor_scalar(
                    tv[:, cls_i, 0:fd], tg[:, 0:fd], float(c), None, Alu.is_equal
                )
            a12 = pool_t2.tile([p, 2, fdmax], fp16, tag="a12")
            nc.vector.tensor_tensor(
                a12[:, :, 0:fd], tv[:, 0:2, 0:fd], ev[:, 0:2, 0:fd], Alu.mult
            )

            for c in order:
                cls_i = c - 1  # 0..3
                on_pe = cls_i >= 2
                e = ev[:, cls_i]
                t_c = tv[:, cls_i]
                if on_pe:
                    # inter_c: accumulate t^T e blocks on PE (fused mult+reduce)
                    first = i == 0
                    last = i == nt - 1
                    nchunks = fd // 128
                    for k in range(nchunks):
                        o = k * 128
                        nc.tensor.matmul(
                            cm[cls_i - 2],
                            t_c[:, o : o + 128],
                            e[:, o : o + 128],
                            start=(first and k == 0),
                            stop=(last and k == nchunks - 1),
                        )
                else:
                    if i < pe_tsum_from:
                        # inter cols q=cls_i on ACT (early tiles)
                        nc.scalar.activation(
                            dump[:, 0:fd],
                            a12[:, cls_i, 0:fd],
                            Act.Copy,
                            accum_out=acc[:, cls_i * nt + i : cls_i * nt + i + 1],
                        )
                    else:
                        for k, (o, w) in enumerate(chunk_list(fd, ts_w)):
                            nc.tensor.matmul(
                                tsp[4 + cls_i][:, 0:w],
                                ones,
                                a12[:, cls_i, o : o + w],
                                start=(i == pe_tsum_from and k == 0),
                                stop=(i == nt - 1 and o + w == fd),
                            )
                if i < pe_tsum_from:
                    # tsum cols q=2+cls_i on ACT (early tiles)
                    nc.scalar.activation(
                        dump[:, 0:fd],
                        t_c[:, 0:fd],
                        Act.Copy,
                        accum_out=acc[
                            :, (2 + cls_i) * nt + i : (2 + cls_i) * nt + i + 1
                        ],
                    )
                else:
                    # late tiles: tsum via PE ones-matmul so ACT drains early
                    for k, (o, w) in enumerate(chunk_list(fd, ts_w)):
                        nc.tensor.matmul(
                            tsp[cls_i][:, 0:w],
                            ones,
                            t_c[:, o : o + w],
                            start=(i == pe_tsum_from and k == 0),
                            stop=(
                                i == nt - 1 and o + w == fd
                            ),
                        )

        nc.sync.dma_start(out=partials_ap, in_=acc)
        # PSUM is not DMA-able: stage the confusion blocks (DVE) and the
        # tsum rows (ACT) through SBUF.
        cmout = pool_acc.tile([p, 256], f32, tag="cmout")
        nc.vector.tensor_copy(cmout[:, 0:128], cm[0])
        nc.vector.tensor_copy(cmout[:, 128:256], cm[1])
        nc.sync.dma_start(out=cms_ap, in_=cmout)
        if pe_tsum_from < nt:
            # split the staging copies across DVE and ACT (both idle here)
            tsout = pool_acc.tile([1, 6 * ts_w], f32, tag="tsout")
            for q in range(6):
                dst = tsout[:, q * ts_w : (q + 1) * ts_w]
                if q % 2 == 0:
                    nc.vector.tensor_copy(dst, tsp[q])
                else:
                    nc.scalar.activation(dst, tsp[q], Act.Copy)
            nc.sync.dma_start(out=tsums_ap, in_=tsout)


_PROGRAM_CACHE = {}


def build_program():
    key = (C, P, FTOT, tuple(TILES))
    if key in _PROGRAM_CACHE:
        return _PROGRAM_CACHE[key]
    nc = bacc.Bacc("TRN2", debug=False, target_bir_lowering=False)
    logits = nc.dram_tensor(
        "logits", [C, P, FTOT], mybir.dt.float16, kind="ExternalInput"
    )
    tgt = nc.dram_tensor("tgt", [P, FTOT], mybir.dt.float16, kind="ExternalInput")
    partials = nc.dram_tensor(
        "partials", [P, 6 * NT], mybir.dt.float32, kind="ExternalOutput"
    )
    cms = nc.dram_tensor("cms", [P, 256], mybir.dt.float32, kind="ExternalOutput")
    tsums = nc.dram_tensor(
        "tsums", [1, 6 * 512], mybir.dt.float32, kind="ExternalOutput"
    )
    with tile.TileContext(nc) as tc:
        emit_dice_kernel(
            tc,
            logits.ap(),
            tgt.ap(),
            partials.ap(),
            cms.ap(),
            NCLS,
            P,
            TILES,
            pe_tsum_from=PE_TSUM_FROM,
            tsums_ap=tsums.ap(),
        )
    nc.compile()
    _PROGRAM_CACHE[key] = nc
    return nc


def make_in_maps(input2, target1):
    lg16 = np.asarray(input2, dtype=np.float32).astype(np.float16)
    tg16 = np.asarray(target1).astype(np.float16)
    lgf = lg16.reshape(B, C, NVOX // B)
    tgf = tg16.reshape(B, NVOX // B)
    shards_per_b = N_CORES // B
    s = (NVOX // B) // shards_per_b
    in_maps = []
    for core in range(N_CORES):
        b, q = divmod(core, shards_per_b)
        sl = slice(q * s, (q + 1) * s)
        in_maps.append(
            {
                "logits": np.ascontiguousarray(lgf[b, :, sl]).reshape(C, P, FTOT),
                "tgt": np.ascontiguousarray(tgf[b, sl]).reshape(P, FTOT),
            }
        )
    return in_maps


def _finish(results):
    """Host-side reduction of per-core partials -> scalar loss (float32).

    partials [P, 6*NT]: cols q*NT+i, q in {inter_1, inter_2, tsum_1..tsum_4};
    cms [P, 256]: accumulated t^T e blocks for classes 3, 4 -- trace = inter.
    """
    total = np.zeros(NQ, dtype=np.float64)  # inter_1..4, tsum_1..4
    for r in results:
        pa = r["partials"].astype(np.float64).reshape(P, 6, NT).sum(axis=(0, 2))
        cms = r["cms"].astype(np.float64)
        ts = r["tsums"].astype(np.float64).reshape(6, 512).sum(axis=1)
        total[0] += pa[0] + ts[4]  # inter_1
        total[1] += pa[1] + ts[5]  # inter_2
        total[2] += np.trace(cms[:, 0:128])  # inter_3
        total[3] += np.trace(cms[:, 128:256])  # inter_4
        total[4] += pa[2] + ts[0]  # tsum_1
        total[5] += pa[3] + ts[1]  # tsum_2
        total[6] += pa[4] + ts[2]  # tsum_3
        total[7] += pa[5] + ts[3]  # tsum_4
    inter = total[:NCLS].astype(np.float32)
    tsum = total[NCLS:].astype(np.float32)
    eps = np.float32(EPS)
    dice = (np.float32(2.0) * inter + eps) / (inter + tsum + eps)
    loss = np.float32(1.0) - np.mean(dice, dtype=np.float32)
    return np.array([loss], dtype=np.float32)


# test.py can set e.g. RUN_KWARGS.update(trace=True) to profile; the grader
# path leaves this empty.
RUN_KWARGS = {}
LAST_RESULT = None


def kernel(input2, target1):
    global LAST_RESULT
    nc = build_program()
    in_maps = make_in_maps(input2, target1)
    res = run_bass_kernel_spmd(nc, in_maps, core_ids=list(range(N_CORES)), **RUN_KWARGS)
    LAST_RESULT = res
    return _finish(res.results)
